# revision 58
# baseline (speedup 1.0000x reference)
"""DecoderOnlyAFT Trainium2 kernel: build + host prep/unshard.

Sharding: 8 cores = 4 batches x 2 sequence-halves. Core c -> (b=c//2, half=c%2).
Each core owns 512 tokens; buffer has 640 cols = 64 halo | 512 owned | 64 pad.
Activations feature-major [128 d-part, d-chunk, token-col], all bf16; k/v/ek/ekv
token-major bf16. emb (= h + pos) is the resident state: LN2's epilogue writes
emb_{l+1} directly via a host-precomputed (pos + ln_b) tensor. The embedding
lookup itself (tok_emb[x]*scale + pos) is host prep, like the exp(w_bias) table.
The 64-token halo is exchanged per layer as bf16 right after LN2 on the right
half, hiding the collective under ~1 layer of compute.

PE.SEQ dispatch (~170ns per LDW+MM pair) is a co-bottleneck with the PE array,
so the big GEMMs (S0/S2/S3, kv) run full-width N=512 moving operands; only q
and the LN stages are column-halved for software pipelining. All ACT functions
({exp, ln, relu, square, copy}) are forced into one table set
(natural_log_exp_and_others) so there are no mid-kernel table reloads;
rsqrt(v) is computed as exp(-0.5*ln(v+eps)).
"""
import sys
sys.path.insert(0, '/opt/trn_rl_repo')
import numpy as np
import concourse.bass as bass
import concourse.mybir as mybir
import concourse.tile as tile
from concourse import bacc
from concourse.hw_specs import get_activation_tables

F32 = mybir.dt.float32
BF16 = mybir.dt.bfloat16
AF = mybir.ActivationFunctionType
ALU = mybir.AluOpType

L, D, H, V, T, S_WIN = 6, 512, 2048, 32000, 1024, 64
NB = 640            # buffer cols: 64 halo | 512 owned | 64 pad
NOWN = 512
KC = 4              # d chunks of 128
HC = 16             # hidden chunks of 128
UB = 5              # token-major u-blocks of 128
TB = 4              # owned t-blocks of 128
EPS = 1e-5
SCALE = 1.0 / np.sqrt(np.float32(D))
GROUPS = [[0, 1], [2, 3], [4, 5], [6, 7]]

_ACT_SET = "natural_log_exp_and_others"


def _pin_act_table(arch):
    """Shrink every other ACT table set so the insert_act_table_loads pass has
    exactly one candidate set for {exp, ln, relu, square, copy, identity} and
    never thrashes between sets. Keys/indices stay intact (ids must match
    act_info.json ordering); only the cached contents are narrowed."""
    tabs = get_activation_tables(arch)
    keep = tabs[_ACT_SET]
    ours = {AF.Exp, AF.Ln, AF.Relu, AF.Square, AF.Copy, AF.Identity}
    for name, s in tabs.items():
        if name != _ACT_SET:
            s -= ours
    assert ours <= keep


def build(use_cc=True, mm_bf16=True, dbg=False, dbg_layer=0):
    nc = bacc.Bacc("TRN2", target_bir_lowering=False, debug=False, num_devices=8)
    _pin_act_table(nc.m.arch)
    dbg_d = {}
    if dbg:
        for nm, shp in [('dsq', [128, KC, NOWN]), ('dek', [128, UB, D]),
                        ('dekv', [128, UB, D]), ('dym', [128, KC, NOWN]),
                        ('dx1', [128, KC, NOWN]), ('dat', [128, KC, NOWN]),
                        ('dx2', [128, KC, NOWN]), ('demb1', [128, KC, NB]),
                        ('df1', [128, HC, NOWN]), ('demb0', [128, KC, NB])]:
            dbg_d[nm] = nc.dram_tensor(nm, shp, BF16, kind="ExternalOutput")

    emb0_d = nc.dram_tensor("emb0", [128, KC, NB], BF16, kind="ExternalInput")
    posb_d = nc.dram_tensor("posb", [L, 128, KC, NOWN], BF16, kind="ExternalInput")
    wq_d = nc.dram_tensor("wq", [L, 128, KC, D], BF16, kind="ExternalInput")
    wk_d = nc.dram_tensor("wk", [L, 128, KC, D], BF16, kind="ExternalInput")
    wv_d = nc.dram_tensor("wv", [L, 128, KC, D], BF16, kind="ExternalInput")
    wo_d = nc.dram_tensor("wo", [L, 128, KC, D], BF16, kind="ExternalInput")
    w1_d = nc.dram_tensor("w1", [L, 128, KC, H], BF16, kind="ExternalInput")
    w2_d = nc.dram_tensor("w2", [L, 128, HC, D], BF16, kind="ExternalInput")
    ewt_d = nc.dram_tensor("ewt", [L, 128, TB, 2, 128], BF16, kind="ExternalInput")
    bv_d = nc.dram_tensor("bv", [L, 1, D], BF16, kind="ExternalInput")
    sv_d = nc.dram_tensor("sv", [L, 128, 5, KC], F32, kind="ExternalInput")  # -bq,bo,b2,g,b
    b1_d = nc.dram_tensor("b1", [L, 128, HC], F32, kind="ExternalInput")
    out_d = nc.dram_tensor("out", [128, KC, NOWN], F32, kind="ExternalOutput")

    OW = 64  # owned col offset in buffer

    with tile.TileContext(nc) as tc:
        with (
            tc.tile_pool(name="const", bufs=1) as cpool,
            tc.tile_pool(name="hpool", bufs=1) as hpool,
            tc.tile_pool(name="wsm", bufs=2) as wsm,        # wq..wo, ewt (per-layer, dbuf)
            tc.tile_pool(name="wstream", bufs=4) as wstream,  # w1/w2 chunks
            tc.tile_pool(name="act", bufs=1) as apool,      # per-layer activations
            tc.tile_pool(name="act1", bufs=1) as a1pool,    # f1, hf
            tc.tile_pool(name="lnp", bufs=3) as lnp,        # [128,KC,256] LN scratch
            tc.tile_pool(name="misc", bufs=2) as misc,
            tc.tile_pool(name="psmm", bufs=4, space="PSUM") as psmm,
            tc.tile_pool(name="psband", bufs=2, space="PSUM") as psband,
            tc.tile_pool(name="pslns", bufs=1, space="PSUM") as pslns,
            tc.tile_pool(name="psbcp", bufs=1, space="PSUM") as psbcp,
            tc.tile_pool(name="dram", bufs=2, space="DRAM") as dpool,
        ):
            onesb = cpool.tile([128, 1], BF16)
            nc.vector.memset(onesb[:], 1.0)
            ones1 = cpool.tile([1, 128], BF16)
            nc.vector.memset(ones1[:], 1.0)
            epst = cpool.tile([1, 1], F32)
            nc.vector.memset(epst[:], EPS)

            embm = hpool.tile([128, KC, NB], BF16, tag="embm")
            nc.sync.dma_start(out=embm[:, :, 320:], in_=emb0_d[:, :, 320:])
            nc.sync.dma_start(out=embm[:, :, 0:320], in_=emb0_d[:, :, 0:320])

            for l in range(L):
                # ---- per-layer weights (double-buffered pools -> prefetch)
                wk = wsm.tile([128, KC, D], BF16, tag="wk")
                nc.sync.dma_start(out=wk[:], in_=wk_d[l])
                wq = wsm.tile([128, KC, D], BF16, tag="wq")
                nc.sync.dma_start(out=wq[:], in_=wq_d[l])
                wv = wsm.tile([128, KC, D], BF16, tag="wv")
                nc.sync.dma_start(out=wv[:], in_=wv_d[l])
                wo = wsm.tile([128, KC, D], BF16, tag="wo")
                nc.sync.dma_start(out=wo[:], in_=wo_d[l])
                ewt = wsm.tile([128, TB, 2, 128], BF16, tag="ewt")
                nc.sync.dma_start(out=ewt[:], in_=ewt_d[l])
                bv = misc.tile([1, D], BF16, tag="bv")
                nc.sync.dma_start(out=bv[:], in_=bv_d[l])
                sv = misc.tile([128, 5, KC], F32, tag="sv")
                nc.sync.dma_start(out=sv[:], in_=sv_d[l])
                b1 = misc.tile([128, HC], F32, tag="b1")
                nc.sync.dma_start(out=b1[:], in_=b1_d[l])
                posb = misc.tile([128, KC, NOWN], BF16, tag="posb")
                nc.sync.dma_start(out=posb[:], in_=posb_d[l])
                w1cs, w2cs = [], []
                for hq in range(KC):
                    w1c = wstream.tile([128, KC, 512], BF16, tag="w1c", bufs=5,
                                       name=f"w1c_{l}_{hq}")
                    nc.sync.dma_start(out=w1c[:], in_=w1_d[l][:, :, 512 * hq:512 * hq + 512])
                    w1cs.append(w1c)
                for j in range(KC):
                    w2c = wstream.tile([128, HC, 128], BF16, tag="w2c", bufs=8,
                                       name=f"w2c_{l}_{j}")
                    nc.sync.dma_start(out=w2c[:], in_=w2_d[l][:, :, 128 * j:128 * j + 128])
                    w2cs.append(w2c)

                sq = apool.tile([128, KC, NOWN], BF16, tag="sq")
                ek = apool.tile([128, UB, D], BF16, tag="ek")
                ekv = apool.tile([128, UB, D], BF16, tag="ekv")
                ym = apool.tile([128, KC, NOWN], BF16, tag="ym")
                x1 = apool.tile([128, KC, NOWN], BF16, tag="x1")
                attnm = apool.tile([128, KC, NOWN], BF16, tag="attnm")
                x2 = apool.tile([128, KC, NOWN], BF16, tag="x2")
                f1 = a1pool.tile([128, HC, NOWN], BF16, tag="f1")
                hf = None
                if l == L - 1:
                    hf = a1pool.tile([128, KC, NOWN], F32, tag="hf", name="hf")

                def qf(ch):  # sq = exp(-(q + bq)); sigmoid folded into den later
                    cs = slice(OW + 256 * ch, OW + 256 * ch + 256)
                    for p in range(2):
                        ps = psmm.tile([128, 2, 256], F32, tag="mm", name=f"psq{l}_{ch}_{p}")
                        for jj in range(2):
                            j = 2 * p + jj
                            for kc in range(KC):
                                nc.tensor.matmul(ps[:, jj, :], wq[:, kc, 128 * j:128 * j + 128],
                                                 embm[:, kc, cs], start=(kc == 0),
                                                 stop=(kc == KC - 1))
                        for jj in range(2):
                            j = 2 * p + jj
                            nc.scalar.activation(sq[:, j, 256 * ch:256 * ch + 256],
                                                 ps[:, jj, :], AF.Exp, scale=-1.0,
                                                 bias=sv[:, 0, j:j + 1])

                def kvf(ub):  # ek/ekv for one u-block; bk dropped (cancels in num/den)
                    ts = slice(128 * ub, 128 * ub + 128)
                    psk = psmm.tile([128, D], F32, tag="mm", name=f"psk{l}_{ub}")
                    for kc in range(KC):
                        nc.tensor.matmul(psk[:], embm[:, kc, ts], wk[:, kc, :],
                                         start=(kc == 0), stop=(kc == KC - 1))
                    nc.scalar.activation(ek[:, ub, :], psk[:], AF.Exp)
                    psv = psmm.tile([128, D], F32, tag="mm", name=f"psv{l}_{ub}")
                    for kc in range(KC):
                        nc.tensor.matmul(psv[:], embm[:, kc, ts], wv[:, kc, :],
                                         start=(kc == 0), stop=False)
                    nc.tensor.matmul(psv[:], ones1[:], bv[:], start=False, stop=True)
                    nc.vector.tensor_tensor(ekv[:, ub, :], ek[:, ub, :], psv[:], op=ALU.mult)

                def bandf(i):  # AFT: ym = num * (sig(q)/den), sigmoid folded in
                    psn = psband.tile([128, KC, 128], F32, tag="band", name=f"psn{l}_{i}")
                    psd = psband.tile([128, KC, 128], F32, tag="band", name=f"psd{l}_{i}")
                    for dc in range(KC):
                        ds = slice(128 * dc, 128 * dc + 128)
                        for sb in range(2):
                            nc.tensor.matmul(psn[:, dc, :], ekv[:, i + sb, ds],
                                             ewt[:, i, sb, :], start=(sb == 0), stop=(sb == 1))
                    for dc in range(KC):
                        ds = slice(128 * dc, 128 * dc + 128)
                        for sb in range(2):
                            nc.tensor.matmul(psd[:, dc, :], ek[:, i + sb, ds],
                                             ewt[:, i, sb, :], start=(sb == 0), stop=(sb == 1))
                    ti = slice(128 * i, 128 * i + 128)
                    rden = lnp.tile([128, KC, 128], F32, tag="rden", bufs=3,
                                    name=f"rden{l}_{i}")
                    nc.vector.scalar_tensor_tensor(rden[:], sq[:, :, ti], 1.0, psd[:],
                                                   op0=ALU.add, op1=ALU.mult)
                    nc.vector.reciprocal(rden[:], rden[:])
                    nc.vector.tensor_tensor(ym[:, :, ti], psn[:], rden[:], op=ALU.mult)

                def S0(ch):  # Wo + bias + residual -> x1 (256-col half)
                    os = slice(256 * ch, 256 * ch + 256)
                    for p in range(2):
                        ps = psmm.tile([128, 2, 256], F32, tag="mm", name=f"ps0{l}_{ch}_{p}")
                        for jj in range(2):
                            j = 2 * p + jj
                            for kc in range(KC):
                                nc.tensor.matmul(ps[:, jj, :], wo[:, kc, 128 * j:128 * j + 128],
                                                 ym[:, kc, os], start=(kc == 0),
                                                 stop=(kc == KC - 1))
                        for jj in range(2):
                            j = 2 * p + jj
                            nc.vector.scalar_tensor_tensor(
                                x1[:, j, os], ps[:, jj, :], sv[:, 1, j:j + 1],
                                embm[:, j, OW + 256 * ch:OW + 256 * ch + 256],
                                op0=ALU.add, op1=ALU.add)

                def S1a(ch):
                    return ln_stats(nc, lnp, pslns, misc, onesb, epst, x1,
                                    256 * ch, 256, f"ln1_{l}_{ch}")

                def S1b(ch, stb):  # LN1 apply -> attnm (outs alternate DVE/POOL)
                    tn = ln_apply(nc, lnp, psbcp, misc, ones1, x1, 256 * ch, 256,
                                  stb, f"ln1_{l}_{ch}")
                    os = slice(256 * ch, 256 * ch + 256)
                    for kc in range(KC):
                        eng = nc.vector if kc % 2 == 0 else nc.gpsimd
                        eng.tensor_scalar(attnm[:, kc, os], tn[:, kc, :],
                                          sv[:, 3, kc:kc + 1], sv[:, 4, kc:kc + 1],
                                          op0=ALU.mult, op1=ALU.add)

                def S2():  # FF1 -> f1 (full width; relu+bias evictions on ACT)
                    for hc in range(HC):
                        ps = psmm.tile([128, NOWN], F32, tag="mm", name=f"ps2{l}_{hc}")
                        wsl = w1cs[hc // 4][:, :, 128 * (hc % 4):128 * (hc % 4) + 128]
                        for kc in range(KC):
                            nc.tensor.matmul(ps[:], wsl[:, kc, :], attnm[:, kc, :],
                                             start=(kc == 0), stop=(kc == KC - 1))
                        nc.scalar.activation(f1[:, hc, :], ps[:], AF.Relu,
                                             bias=b1[:, hc:hc + 1])

                def S3():  # FF2 + bias + residual -> x2 (full width)
                    for j in range(KC):
                        ps = psmm.tile([128, NOWN], F32, tag="mm", name=f"ps3{l}_{j}")
                        for hc in range(HC):
                            nc.tensor.matmul(ps[:], w2cs[j][:, hc, :], f1[:, hc, :],
                                             start=(hc == 0), stop=(hc == HC - 1))
                        nc.vector.scalar_tensor_tensor(
                            x2[:, j, :], ps[:], sv[:, 2, j:j + 1],
                            attnm[:, j, :], op0=ALU.add, op1=ALU.add)

                def S4a(c0, cw):
                    return ln_stats(nc, lnp, pslns, misc, onesb, epst, x2,
                                    c0, cw, f"ln2_{l}_{c0}")

                def S4b(c0, cw, stb, pool_outs):
                    # LN2 apply -> emb_{l+1} (pos+b folded); last layer -> hf.
                    # POOL doesn't support scalar_tensor_tensor, so the
                    # epilogue is always on DVE (pool_outs kept for emission
                    # symmetry/documentation).
                    tn = ln_apply(nc, lnp, psbcp, misc, ones1, x2, c0, cw,
                                  stb, f"ln2_{l}_{c0}")
                    os = slice(c0, c0 + cw)
                    for kc in range(KC):
                        dst = hf[:, kc, os] if l == L - 1 else \
                            embm[:, kc, OW + c0:OW + c0 + cw]
                        nc.vector.scalar_tensor_tensor(
                            dst, tn[:, kc, :], sv[:, 3, kc:kc + 1],
                            posb[:, kc, os], op0=ALU.mult, op1=ALU.add)
                    if l == L - 1:
                        nc.sync.dma_start(out=out_d[:, :, os], in_=hf[:, :, os])

                def halo_ag():
                    if l < L - 1 and use_cc:
                        ccin = dpool.tile([128, KC, 64], BF16, tag="ccin")
                        ccout = dpool.tile([256, KC, 64], BF16, tag="ccout")
                        nc.sync.dma_start(out=ccin[:], in_=embm[:, :, 512:576])
                        nc.gpsimd.collective_compute(
                            "AllGather", ALU.bypass,
                            replica_groups=GROUPS,
                            ins=[ccin.opt()], outs=[ccout.opt()],
                        )
                        nc.sync.dma_start(out=embm[:, :, 0:64], in_=ccout[0:128])

                if dbg and l == dbg_layer:
                    emb_snap = a1pool.tile([128, KC, NB], BF16, tag="esnap",
                                           name=f"esnap{l}")
                    nc.vector.tensor_copy(emb_snap[:], embm[:])

                # ---- emission order (pipeline order). LN2 runs in three
                # column slices so the 64-col halo source is normalized and
                # exchanged as early as possible; post-collective applies stay
                # off POOL (the collective occupies that queue for ~6us).
                kvf(4); qf(1); kvf(3)
                bandf(3)
                qf(0); kvf(2)
                bandf(2)
                kvf(1)
                bandf(1)
                S0(1)
                st11 = S1a(1)
                kvf(0)
                bandf(0)
                S0(0)
                S1b(1, st11)
                st10 = S1a(0)
                S1b(0, st10)
                S2()
                S3()
                if l < L - 1:
                    st4c = S4a(448, 64)
                    st4b = S4a(256, 192)
                    st40 = S4a(0, 256)
                    S4b(448, 64, st4c, pool_outs=True)
                    halo_ag()
                    S4b(256, 192, st4b, pool_outs=False)
                    S4b(0, 256, st40, pool_outs=False)
                else:
                    sls = [(384, 128), (256, 128), (128, 128), (0, 128)]
                    sts = [S4a(c0, cw) for c0, cw in sls]
                    for (c0, cw), stx in zip(sls, sts):
                        S4b(c0, cw, stx, pool_outs=(c0 >= 256))

                if dbg and l == dbg_layer:
                    nc.sync.dma_start(out=dbg_d['demb0'][:], in_=emb_snap[:])
                    nc.sync.dma_start(out=dbg_d['dsq'][:], in_=sq[:])
                    nc.sync.dma_start(out=dbg_d['dek'][:], in_=ek[:])
                    nc.sync.dma_start(out=dbg_d['dekv'][:], in_=ekv[:])
                    nc.sync.dma_start(out=dbg_d['dym'][:], in_=ym[:])
                    nc.sync.dma_start(out=dbg_d['dx1'][:], in_=x1[:])
                    nc.sync.dma_start(out=dbg_d['dat'][:], in_=attnm[:])
                    nc.sync.dma_start(out=dbg_d['dx2'][:], in_=x2[:])
                    nc.sync.dma_start(out=dbg_d['demb1'][:], in_=embm[:])
                    nc.sync.dma_start(out=dbg_d['df1'][:], in_=f1[:])

    nc.compile()
    return nc


def ln_stats(nc, lnp, pslns, misc, onesb, epst, x, c0, cw, nm):
    """LN stats over the partition(d) axis for owned cols [c0, c0+cw).
    Returns stb = [rstd, mu] (bf16, [1,2,cw]). rsqrt is exp(-0.5*ln(var+eps))
    so ACT stays in one function-table set."""
    os = slice(c0, c0 + cw)
    lns = pslns.tile([1, 2, cw], F32, tag="lnsum", name=f"lns_{nm}")
    for kc in range(KC):
        nc.tensor.matmul(lns[:, 0, :], onesb[:], x[:, kc, os], start=(kc == 0),
                         stop=(kc == KC - 1))
    xsq = lnp.tile([128, KC, cw], BF16, tag="lnsq", name=f"xsq_{nm}")
    nc.vector.tensor_tensor(xsq[:], x[:, :, os], x[:, :, os], op=ALU.mult)
    for kc in range(KC):
        nc.tensor.matmul(lns[:, 1, :], onesb[:], xsq[:, kc, :], start=(kc == 0),
                         stop=(kc == KC - 1))
    st = misc.tile([1, 4, cw], F32, tag="lnst", bufs=2, name=f"st_{nm}")
    mu2, ve, lnt = st[:, 0, :], st[:, 1, :], st[:, 2, :]
    stb = misc.tile([1, 2, cw], BF16, tag="stb", bufs=3, name=f"stb_{nm}")
    nc.vector.tensor_scalar_mul(stb[:, 1, :], lns[:, 0, :], 1.0 / D)        # mu (bf16)
    nc.scalar.activation(mu2, lns[:, 0, :], AF.Square, scale=1.0 / D)       # mu^2
    nc.vector.scalar_tensor_tensor(ve, lns[:, 1, :], 1.0 / D, mu2,
                                   op0=ALU.mult, op1=ALU.subtract)          # var
    nc.scalar.activation(lnt, ve, AF.Ln, bias=epst[:])                      # ln(var+eps)
    nc.scalar.activation(stb[:, 0, :], lnt, AF.Exp, scale=-0.5)             # rstd (bf16)
    return stb


def ln_apply(nc, lnp, psbcp, misc, ones1, x, c0, cw, stb, nm):
    """Broadcast [rstd, mu] across partitions via PE, tn = (x - mu) * rstd."""
    os = slice(c0, c0 + cw)
    psbc = psbcp.tile([128, 2, cw], F32, tag="bc", name=f"bc_{nm}")
    nc.tensor.matmul(psbc[:], ones1[:], stb[:], start=True, stop=True)
    rb = misc.tile([128, 2, cw], BF16, tag="rb", bufs=2, name=f"rb_{nm}")
    nc.scalar.activation(rb[:], psbc[:], AF.Copy)
    tc_t = lnp.tile([128, KC, cw], BF16, tag="lnsq", name=f"tc_{nm}")
    nc.vector.tensor_tensor(tc_t[:], x[:, :, os],
                            rb[:, 1:2, :].to_broadcast([128, KC, cw]), op=ALU.subtract)
    tn = lnp.tile([128, KC, cw], BF16, tag="lnsq", name=f"tn_{nm}")
    nc.vector.tensor_tensor(tn[:], tc_t[:],
                            rb[:, 0:1, :].to_broadcast([128, KC, cw]), op=ALU.mult)
    return tn


# ------------------------- host side -------------------------

def prep_inputs(inputs, mm_bf16=True):
    """inputs: dict from setup_inputs(). Returns per_core list of input dicts."""
    x = np.asarray(inputs['x']).astype(np.int64)
    toke = np.asarray(inputs['tok_emb'], np.float32)
    pose = np.asarray(inputs['pos_emb'], np.float32)
    wb = np.asarray(inputs['w_bias'], np.float32)

    def fm(w, chunks):  # [d_in, n] -> [128, chunks, n]
        return np.ascontiguousarray(w.reshape(chunks, 128, -1).transpose(1, 0, 2))

    import ml_dtypes
    mdt = ml_dtypes.bfloat16
    wq = np.stack([fm(np.asarray(inputs['Wq'][l], np.float32), KC) for l in range(L)])
    wk = np.stack([fm(np.asarray(inputs['Wk'][l], np.float32), KC) for l in range(L)])
    wv = np.stack([fm(np.asarray(inputs['Wv'][l], np.float32), KC) for l in range(L)])
    wo = np.stack([fm(np.asarray(inputs['Wo'][l], np.float32), KC) for l in range(L)])
    w1 = np.stack([fm(np.asarray(inputs['W1'][l], np.float32), KC) for l in range(L)])
    w2 = np.stack([fm(np.asarray(inputs['W2'][l], np.float32), HC) for l in range(L)])

    bv = np.asarray(inputs['bv'], np.float32).reshape(L, 1, D)

    def pv(name):  # per-d vector [L, D] -> [L, 128, KC]
        v = np.asarray(inputs[name], np.float32)
        return v.reshape(L, KC, 128).transpose(0, 2, 1)

    sv = np.stack([-pv('bq'), pv('bo'), pv('b2'), pv('ln_g'), pv('ln_b')], axis=2)
    sv = np.ascontiguousarray(sv)  # [L, 128, 5, KC]
    b1 = np.ascontiguousarray(
        np.asarray(inputs['b1'], np.float32).reshape(L, HC, 128).transpose(0, 2, 1))

    # ew per layer (global, fp32, matches reference math)
    t = np.arange(T)
    mask = (t[:, None] >= t[None, :]) & (t[:, None] - t[None, :] < S_WIN)
    NEG = np.float32(-1e30)
    ew_all = []
    for l in range(L):
        wm = np.where(mask, wb[l], NEG).astype(np.float32)
        wm = wm - wm.max(axis=1, keepdims=True)
        ew_all.append(np.exp(wm).astype(np.float32))

    lnb_fm = pv('ln_b')  # [L, 128, KC]

    per_core = []
    for c in range(8):
        b, half = c // 2, c % 2
        base = half * 512
        # emb0 = tok_emb[x]*scale + pos*scale, feature-major over buffer cols
        g = base - 64 + np.arange(NB)
        ok = (g >= 0) & (g < T)
        # reference: h = tok*s + pos*s, then each layer uses emb = h + pos,
        # so layer 0's AFT input carries pos TWICE.
        e0 = np.zeros((NB, D), np.float32)
        e0[ok] = toke[x[b, g[ok]]] * SCALE + 2.0 * pose[g[ok]] * SCALE
        emb0 = np.ascontiguousarray(e0.T.reshape(KC, 128, NB).transpose(1, 0, 2))
        # posb [L, 128, KC, NOWN]: pos_own + ln_b (last layer: ln_b only)
        pos_own = (pose[base:base + NOWN] * SCALE).T.reshape(KC, 128, NOWN)
        pos_own = pos_own.transpose(1, 0, 2)  # [128, KC, NOWN]
        posb = np.zeros((L, 128, KC, NOWN), np.float32)
        for l in range(L):
            posb[l] = lnb_fm[l][:, :, None]
            if l < L - 1:
                posb[l] += pos_own
        # ewt [L, 128, TB, 2, 128]
        ewt = np.zeros((L, 128, TB, 2, 128), np.float32)
        for l in range(L):
            ew = ew_all[l]
            for i in range(TB):
                gt0 = base + 128 * i
                for s in range(2):
                    gu0 = base - 64 + 128 * (i + s)
                    u0, u1 = max(0, gu0), min(T, gu0 + 128)
                    if u1 <= u0:
                        continue
                    sub = ew[gt0:gt0 + 128, u0:u1]     # [t, u]
                    ewt[l, u0 - gu0:u1 - gu0, i, s, :] = sub.T
        per_core.append(dict(
            emb0=emb0.astype(mdt), posb=posb.astype(mdt),
            wq=wq.astype(mdt), wk=wk.astype(mdt), wv=wv.astype(mdt),
            wo=wo.astype(mdt), w1=w1.astype(mdt), w2=w2.astype(mdt),
            ewt=np.ascontiguousarray(ewt).astype(mdt), bv=bv.astype(mdt),
            sv=sv, b1=b1,
        ))
    return per_core


def unshard(results):
    """results: list of 8 dicts with 'out' [128, KC, NOWN] -> [4, T, D]."""
    full = np.zeros((4, T, D), np.float32)
    for c in range(8):
        b, half = c // 2, c % 2
        o = results[c]['out']  # [128, KC, 512]
        full[b, half * 512:(half + 1) * 512, :] = \
            o.transpose(2, 1, 0).reshape(NOWN, D)
    return full


# ------------------------- public entry -------------------------

_NC_CACHE = {}


def _get_nc():
    if 'nc' not in _NC_CACHE:
        _NC_CACHE['nc'] = build(use_cc=True, mm_bf16=True)
    return _NC_CACHE['nc']


def kernel(**inputs) -> np.ndarray:
    """Full-input, full-output DecoderOnlyAFT forward on 8 NeuronCores."""
    from concourse.bass_utils import run_bass_kernel_spmd
    per_core = prep_inputs(inputs, mm_bf16=True)
    nc = _get_nc()
    res = run_bass_kernel_spmd(nc, per_core, core_ids=list(range(8)))
    return unshard(res.results)
